# revision 6
# baseline (speedup 1.0000x reference)
"""I-slice-parallel MoE kernel for Trainium2 (8 NeuronCores), v4.

Instead of expert-parallel (each core = one expert, padded to the max
expert's token count), every core processes ALL experts' gathered tokens
over its own 512-wide slice of the intermediate dimension I:

    core c: for each expert e:
       Hh_e[i_slice, C_e] = silu(G_e[i_slice] X_e^T) * (U_e[i_slice] X_e^T)
       Yp_e^T[H, C_e]    += D_e^T[i_slice] contribution   (partial sum)

Per-core work is identical by construction (sum of exact token counts,
no capacity padding), cutting PE cycles ~6% vs expert-parallel C_max.
The host sums the 8 partial Y^T outputs and scatters per expert.
"""

import sys
from contextlib import ExitStack

if "/opt/trn_rl_repo" not in sys.path:
    sys.path.insert(0, "/opt/trn_rl_repo")

import ml_dtypes
import numpy as np

import concourse.bacc as bacc
import concourse.mybir as mybir
import concourse.tile as tile
from concourse.bass_utils import run_bass_kernel_spmd

B, S, H, I, E, TOPK = 4, 2048, 1024, 4096, 8, 2
T = B * S
KCH = H // 128   # 8 contraction chunks over H
HB = H // 128    # 8 output blocks over H
NIB = 4          # i-blocks per core slice (512 / 128)
ISL = 512        # per-core I slice width
BF16 = mybir.dt.bfloat16
F32 = mybir.dt.float32

_prog_cache: dict[tuple, object] = {}
_last_counts: tuple = ()


def _ctiles(C):
    """Free-dim tiles of <=512 covering [0, C) (no padding)."""
    out = []
    c = 0
    while c < C:
        s = min(512, C - c)
        out.append((c, s))
        c += s
    return out


def build_program(CT, reps=1):
    counts = _last_counts
    assert sum(counts) == CT, (counts, CT)
    key = (counts, reps)
    if key in _prog_cache:
        return _prog_cache[key]
    nc = bacc.Bacc("TRN2", target_bir_lowering=False, debug=False, num_devices=8)

    xt_d = nc.dram_tensor("xt", [128, KCH, CT], BF16, kind="ExternalInput").ap()
    gt_d = nc.dram_tensor("gt", [E * NIB, 128, KCH, 128], BF16, kind="ExternalInput").ap()
    ut_d = nc.dram_tensor("ut", [E * NIB, 128, KCH, 128], BF16, kind="ExternalInput").ap()
    # D^T slice packed [128(p=i%128), E, NIB, H]
    dt_d = nc.dram_tensor("dt", [128, E, NIB, H], BF16, kind="ExternalInput").ap()
    # partial y^T blocks: y_d[hb] = Yp^T[hb*128:(hb+1)*128, :]
    y_d = nc.dram_tensor("y", [HB, 128, CT], BF16, kind="ExternalOutput").ap()

    with tile.TileContext(nc) as tc:
        with ExitStack() as stack:
            if reps > 1:
                stack.enter_context(tc.For_i(0, reps, 1))
            _emit_body(nc, tc, counts, xt_d, gt_d, ut_d, dt_d, y_d)

    nc.compile()
    _prog_cache[key] = nc
    return nc


def _emit_body(nc, tc, counts, xt_d, gt_d, ut_d, dt_d, y_d):
    cmax = max(counts)
    offs = np.concatenate([[0], np.cumsum(counts)]).astype(int)

    with (
        tc.tile_pool(name="wpool", bufs=3) as wpool,
        tc.tile_pool(name="xpool", bufs=2) as xpool,
        tc.tile_pool(name="dpool", bufs=1) as dpool,
        tc.tile_pool(name="hpool", bufs=2) as hpool,
        tc.tile_pool(name="spool", bufs=2) as spool,
        tc.tile_pool(name="ypool", bufs=2) as ypool,
        tc.tile_pool(name="psum", bufs=1, space="PSUM") as psum,
    ):
        # D^T slice resident, split along H so re-loads overlap compute
        dta = dpool.tile([128, E, NIB, H // 2], BF16, tag="dta")
        nc.sync.dma_start(dta[:], dt_d[:, :, :, : H // 2])
        dtb = dpool.tile([128, E, NIB, H // 2], BF16, tag="dtb")
        nc.sync.dma_start(dtb[:], dt_d[:, :, :, H // 2 :])

        for e in range(E):
            Ce = counts[e]
            goff = int(offs[e])
            tiles = _ctiles(Ce)

            # expert-local X^T streamed in (double buffered across experts)
            xs = xpool.tile([128, KCH, cmax], BF16, tag="xt", name="xs")
            nc.sync.dma_start(xs[:, :, :Ce], xt_d[:, :, goff : goff + Ce])

            # ---- stage 1: Hh[i_slice, c] for this expert ----
            hhs = []
            for ib in range(NIB):
                hh = hpool.tile([128, cmax], BF16, tag=f"hh{ib}", name=f"hh{ib}")
                hhs.append(hh)
            for ib in range(NIB):
                gt = wpool.tile([128, KCH, 128], BF16, tag="gt")
                nc.sync.dma_start(gt[:], gt_d[e * NIB + ib])
                ut = wpool.tile([128, KCH, 128], BF16, tag="ut")
                nc.sync.dma_start(ut[:], ut_d[e * NIB + ib])
                for ci, (c0, cs) in enumerate(tiles):
                    a1 = psum.tile([128, cs], F32, tag=f"a1{ci % 2}", name="a1")
                    for k in range(KCH):
                        nc.tensor.matmul(
                            a1[:], gt[:, k, :], xs[:, k, c0 : c0 + cs],
                            start=(k == 0), stop=(k == KCH - 1),
                        )
                    a2 = psum.tile([128, cs], F32, tag=f"a2{ci % 2}", name="a2")
                    for k in range(KCH):
                        nc.tensor.matmul(
                            a2[:], ut[:, k, :], xs[:, k, c0 : c0 + cs],
                            start=(k == 0), stop=(k == KCH - 1),
                        )
                    sl = spool.tile([128, cs], F32, tag=f"sl{ci % 2}", name="sl")
                    nc.scalar.activation(
                        sl[:], a1[:], mybir.ActivationFunctionType.Silu
                    )
                    nc.vector.tensor_mul(hhs[ib][:, c0 : c0 + cs], sl[:], a2[:])

            # ---- stage 2: partial Y^T[h, c] over the 4 resident i-blocks ----
            for hpass, dts in ((0, dta), (1, dtb)):
                for c0, cs in tiles:
                    pys = [
                        psum.tile([128, cs], F32, tag=f"y{j}", name=f"py{j}")
                        for j in range(4)
                    ]
                    for ic in range(NIB):
                        for j in range(4):
                            nc.tensor.matmul(
                                pys[j][:],
                                dts[:, e, ic, j * 128 : (j + 1) * 128],
                                hhs[ic][:, c0 : c0 + cs],
                                start=(ic == 0), stop=(ic == NIB - 1),
                            )
                    for j in range(4):
                        hb = hpass * 4 + j
                        yt = ypool.tile([128, cs], BF16, tag=f"yt{j}", name="yt")
                        nc.scalar.copy(yt[:], pys[j][:])
                        nc.sync.dma_start(
                            y_d[hb][:, goff + c0 : goff + c0 + cs], yt[:]
                        )


def _routing(x, router_w):
    """Replicate the reference's routing decisions with identical jax ops."""
    import jax
    import jax.numpy as jnp

    xf = jnp.asarray(x).reshape(-1, H)
    logits = xf @ jnp.asarray(router_w).T
    probs = jax.nn.softmax(logits, axis=-1)
    topk_p, topk_i = jax.lax.top_k(probs, TOPK)
    topk_p = topk_p / topk_p.sum(axis=-1, keepdims=True)
    return np.asarray(topk_p), np.asarray(topk_i)


def prepare(x, router_w, gate_w, up_w, down_w):
    """Host-side dispatch: returns (nc, in_maps, combine)."""
    global _last_counts
    topk_p, topk_i = _routing(x, router_w)
    xf = np.ascontiguousarray(np.asarray(x, dtype=np.float32).reshape(T, H))

    idxs, weights = [], []
    for e in range(E):
        sel = topk_i == e
        mask = sel.any(axis=-1)
        w_tok = (topk_p * sel).sum(axis=-1)
        cnt = int(mask.sum())
        mean_w = float(w_tok.sum() / max(cnt, 1)) if cnt > 0 else 0.0
        idxs.append(np.nonzero(mask)[0])
        weights.append(np.float32(mean_w))

    counts = tuple(len(ix) for ix in idxs)
    _last_counts = counts
    CT = sum(counts)
    offs = np.concatenate([[0], np.cumsum(counts)]).astype(int)

    xf_bf = xf.astype(ml_dtypes.bfloat16)
    # global X^T: all experts' gathered tokens concatenated [128, KCH, CT]
    xt = np.empty((128, KCH, CT), dtype=ml_dtypes.bfloat16)
    for e in range(E):
        ix = idxs[e]
        xt[:, :, offs[e] : offs[e + 1]] = (
            xf_bf[ix].T.reshape(KCH, 128, len(ix)).transpose(1, 0, 2)
        )

    in_maps = []
    for c in range(E):  # core c owns I-slice [c*512, (c+1)*512)
        gts, uts, dts = [], [], []
        for e in range(E):
            gT = np.asarray(gate_w[e], dtype=np.float32).T.astype(ml_dtypes.bfloat16)
            uT = np.asarray(up_w[e], dtype=np.float32).T.astype(ml_dtypes.bfloat16)
            gs = gT[:, c * ISL : (c + 1) * ISL]  # [H, 512]
            us = uT[:, c * ISL : (c + 1) * ISL]
            gts.append(
                np.ascontiguousarray(
                    gs.reshape(KCH, 128, NIB, 128).transpose(2, 1, 0, 3)
                )
            )
            uts.append(
                np.ascontiguousarray(
                    us.reshape(KCH, 128, NIB, 128).transpose(2, 1, 0, 3)
                )
            )
            dT = np.asarray(down_w[e], dtype=np.float32).T.astype(ml_dtypes.bfloat16)
            ds = dT[c * ISL : (c + 1) * ISL, :]  # [512, H]
            dts.append(ds.reshape(NIB, 128, H).transpose(1, 0, 2))  # [128,NIB,H]
        gt = np.concatenate(gts, axis=0)                    # [E*NIB,128,KCH,128]
        ut = np.concatenate(uts, axis=0)
        dt = np.ascontiguousarray(np.stack(dts, axis=1))    # [128,E,NIB,H]
        in_maps.append({"xt": xt, "gt": gt, "ut": ut, "dt": dt})

    nc = build_program(CT)

    def combine(results):
        ysum = np.zeros((HB, 128, CT), dtype=np.float32)
        for c in range(E):
            ysum += results[c]["y"].astype(np.float32)
        yT = ysum.reshape(H, CT)
        out = np.zeros((T, H), dtype=np.float32)
        for e in range(E):
            ix = idxs[e]
            out[ix] += weights[e] * yT[:, offs[e] : offs[e + 1]].T
        return out.reshape(B, S, H)

    return nc, in_maps, combine


def kernel(x, router_w, gate_w, up_w, down_w):
    nc, in_maps, combine = prepare(x, router_w, gate_w, up_w, down_w)
    res = run_bass_kernel_spmd(nc, in_maps, list(range(8)))
    return combine(res.results)
